# revision 4
# baseline (speedup 1.0000x reference)
"""Trainium2 Bass kernel for masked additive-attention pooling.

Reference math (per batch b):
    whhn = encoding @ W_h.T                            # [B, D]
    M    = tanh(X @ W_y.T + whhn[:, None, :])          # [B, T, D]
    a    = sigmoid(M @ w_a)                            # [B, T]
    e    = exp(a); den = sum(e * mask); w = e * mask / den
    out  = sum_t w[t] * X[t]                           # [B, D]

Sharding: data-parallel over batch B=32 across 8 cores (4 batches/core).
Weights replicated. All layout transforms (transposes of the tiny weight
matrices, column-layout repack of mask/encoding) are done host-side; all
FLOPs run on device.
"""

import sys

if "/opt/trn_rl_repo" not in sys.path:
    sys.path.insert(0, "/opt/trn_rl_repo")

import numpy as np
import ml_dtypes

import concourse.bacc as bacc
import concourse.mybir as mybir
import concourse.tile as tile
from concourse.bass_utils import run_bass_kernel_spmd

F32 = mybir.dt.float32
F32R = mybir.dt.float32r
BF16 = mybir.dt.bfloat16
AF = mybir.ActivationFunctionType

N_CORES = 8
B, T, D = 32, 2048, 1024
B_LOC = B // N_CORES          # 4 batches per core
NTOK = B_LOC * T              # 8192 tokens per core
TILE_T = 512                  # tokens per big tile
NBT = NTOK // TILE_T          # 16 big tiles
BT_PER_B = T // TILE_T        # 4 big tiles per batch
CH = TILE_T // 128            # 4 chunks of 128 tokens per big tile
KD = D // 128                 # 8 contraction chunks
EB = D // 128                 # 8 output-feature blocks

_CACHE = {}


def build():
    nc = bacc.Bacc("TRN2", target_bir_lowering=False, debug=False,
                   num_devices=N_CORES)

    x = nc.dram_tensor("x", [NTOK, D], F32R, kind="ExternalInput").ap()
    wyt = nc.dram_tensor("wyt", [EB * KD, 128, 128], F32R,
                         kind="ExternalInput").ap()
    wht = nc.dram_tensor("wht", [EB * KD, 128, 128], BF16,
                         kind="ExternalInput").ap()
    enc_cols = nc.dram_tensor("enc_cols", [128, KD * B_LOC], BF16,
                              kind="ExternalInput").ap()
    wa_cols = nc.dram_tensor("wa_cols", [128, EB], BF16,
                             kind="ExternalInput").ap()
    mask_cols = nc.dram_tensor("mask_cols", [128, NTOK // 128], F32,
                               kind="ExternalInput").ap()
    ident = nc.dram_tensor("ident", [128, 128], F32R,
                           kind="ExternalInput").ap()
    ones = nc.dram_tensor("ones", [128, 2], F32R, kind="ExternalInput").ap()
    out = nc.dram_tensor("out", [B_LOC, D], F32, kind="ExternalOutput").ap()

    with tile.TileContext(nc) as tc:
        with tc.tile_pool(name="consts", bufs=1) as cp, \
             tc.tile_pool(name="wy", bufs=1) as wyp, \
             tc.tile_pool(name="xnat", bufs=2) as xp, \
             tc.tile_pool(name="xt", bufs=2) as xtp, \
             tc.tile_pool(name="th", bufs=2) as thp, \
             tc.tile_pool(name="small", bufs=3) as smp, \
             tc.tile_pool(name="mps", bufs=1, space="PSUM") as psum:

            # ---- phase 0: load constants / weights, compute whhn^T ----
            def load_x_tile(j):
                tiles = []
                for c in range(CH):
                    t = xp.tile([128, D], F32R, tag=f"x{c}", name=f"x_{j}_{c}")
                    nc.sync.dma_start(t[:], x[j * TILE_T + c * 128:
                                             j * TILE_T + (c + 1) * 128, :])
                    tiles.append(t)
                return tiles

            xn = {0: load_x_tile(0)}

            ident_sb = cp.tile([128, 128], F32R)
            ones_sb = cp.tile([128, 2], F32R)
            wa_sb = cp.tile([128, EB], BF16)
            enc_sb = cp.tile([128, KD * B_LOC], BF16)
            mask_sb = cp.tile([128, NTOK // 128], F32)
            whhn_sb = cp.tile([128, EB * B_LOC], F32)
            nc.sync.dma_start(ident_sb[:], ident[:])
            nc.sync.dma_start(ones_sb[:], ones[:])
            nc.sync.dma_start(wa_sb[:], wa_cols[:])
            nc.sync.dma_start(enc_sb[:], enc_cols[:])
            nc.sync.dma_start(mask_sb[:], mask_cols[:])

            with tc.tile_pool(name="wh", bufs=1) as whp:
                wh_sb = []
                for i in range(EB * KD):
                    t = whp.tile([128, 128], BF16, tag=f"wh{i}")
                    nc.sync.dma_start(t[:], wht[i])
                    wh_sb.append(t)
                wy_sb = []
                for i in range(EB * KD):
                    t = wyp.tile([128, 128], F32R, tag=f"wy{i}")
                    nc.sync.dma_start(t[:], wyt[i])
                    wy_sb.append(t)
                # whhn^T[e, b] accumulated per 128-row e-block
                for eb in range(EB):
                    php = psum.tile([128, B_LOC], F32, tag="apre", bufs=1,
                                    name=f"php_{eb}")
                    for k in range(KD):
                        nc.tensor.matmul(
                            php[:], wh_sb[eb * KD + k][:],
                            enc_sb[:, k * B_LOC:(k + 1) * B_LOC],
                            start=(k == 0), stop=(k == KD - 1))
                    nc.vector.tensor_copy(
                        whhn_sb[:, eb * B_LOC:(eb + 1) * B_LOC], php[:])

            # ---- main loop over big tiles ----
            state = {}

            def emit_tr(j):
                xts = []
                for k in range(KD):
                    trp = psum.tile([128, TILE_T], F32R, tag="tr", bufs=2,
                                    name=f"tr_{j}_{k}")
                    for c in range(CH):
                        nc.tensor.transpose(
                            trp[:, c * 128:(c + 1) * 128],
                            xn[j][c][:, k * 128:(k + 1) * 128],
                            ident_sb[:])
                    xt_t = xtp.tile([128, TILE_T], F32R, tag=f"xt{k}",
                                    name=f"xt_{j}_{k}")
                    nc.vector.tensor_copy(xt_t[:], trp[:])
                    xts.append(xt_t)
                state[("xt", j)] = xts

            def emit_z(j):
                b = j // BT_PER_B
                ths = []
                for eb in range(EB):
                    zp = psum.tile([128, TILE_T], F32, tag="z", bufs=2,
                                   name=f"z_{j}_{eb}")
                    for k in range(KD):
                        nc.tensor.matmul(
                            zp[:], wy_sb[eb * KD + k][:, :],
                            state[("xt", j)][k][:],
                            start=(k == 0), stop=(k == KD - 1))
                    th_t = thp.tile([128, TILE_T], BF16, tag=f"th{eb}",
                                    name=f"th_{j}_{eb}")
                    nc.scalar.activation(
                        th_t[:], zp[:], AF.Tanh,
                        bias=whhn_sb[:, eb * B_LOC + b:eb * B_LOC + b + 1])
                    ths.append(th_t)
                state[("th", j)] = ths

            def emit_apre(j):
                app = psum.tile([128, CH], F32, tag="apre", bufs=1,
                                name=f"apre_{j}")
                for c in range(CH):
                    for eb in range(EB):
                        nc.tensor.matmul(
                            app[:, c:c + 1],
                            state[("th", j)][eb][:, c * 128:(c + 1) * 128],
                            wa_sb[:, eb:eb + 1],
                            start=(eb == 0), stop=(eb == EB - 1))
                sig = smp.tile([128, CH], F32, tag="sig", name=f"sig_{j}")
                nc.scalar.activation(sig[:], app[:], AF.Sigmoid)
                ex = smp.tile([128, CH], F32, tag="ex", name=f"ex_{j}")
                nc.scalar.activation(ex[:], sig[:], AF.Exp)
                ew = smp.tile([128, CH], F32R, tag="ew", name=f"ew_{j}")
                nc.vector.tensor_mul(
                    ew[:], ex[:], mask_sb[:, j * CH:(j + 1) * CH])
                state[("ew", j)] = ew

            def emit_pool(j):
                b = j // BT_PER_B
                if j % BT_PER_B == 0:
                    state[("num", b)] = [
                        psum.tile([1, 512], F32, tag=f"num{dn}", bufs=1,
                                  name=f"num_{b}_{dn}")
                        for dn in range(2)]
                    state[("den", b)] = psum.tile([1, 2], F32, tag="den",
                                                  bufs=1, name=f"den_{b}")
                ew = state[("ew", j)]
                first = j % BT_PER_B == 0
                last = j % BT_PER_B == BT_PER_B - 1
                for c in range(CH):
                    st = first and c == 0
                    sp = last and c == CH - 1
                    for dn in range(2):
                        nc.tensor.matmul(
                            state[("num", b)][dn][:],
                            ew[:, c:c + 1],
                            xn[j][c][:, dn * 512:(dn + 1) * 512],
                            start=st, stop=sp)
                    nc.tensor.matmul(
                        state[("den", b)][:], ew[:, c:c + 1], ones_sb[:],
                        start=st, stop=sp)
                if last:
                    rec = smp.tile([1, 1], F32, tag="rec", name=f"rec_{b}")
                    nc.vector.reciprocal(rec[:], state[("den", b)][:, 0:1])
                    ob = smp.tile([1, D], F32, tag="ob", bufs=2,
                                  name=f"ob_{b}")
                    for dn in range(2):
                        nc.vector.tensor_scalar_mul(
                            ob[:, dn * 512:(dn + 1) * 512],
                            state[("num", b)][dn][:], rec[:])
                    nc.sync.dma_start(out[b:b + 1, :], ob[:])

            for j in range(NBT):
                if j + 1 < NBT:
                    xn[j + 1] = load_x_tile(j + 1)
                emit_tr(j)
                if j > 0:
                    emit_pool(j - 1)
                emit_z(j)
                emit_apre(j)
            emit_pool(NBT - 1)

    nc.compile()
    return nc


def _host_pack(full_input, encoding, mask, W_h, W_y, w_a):
    """Build the per-core input maps (pure layout transforms only)."""
    wyT = np.ascontiguousarray(W_y.T)  # [d, e]
    whT = np.ascontiguousarray(W_h.T)
    wyt_blocks = np.empty((EB * KD, 128, 128), np.float32)
    wht_blocks = np.empty((EB * KD, 128, 128), ml_dtypes.bfloat16)
    for eb in range(EB):
        for k in range(KD):
            wyt_blocks[eb * KD + k] = wyT[k * 128:(k + 1) * 128,
                                          eb * 128:(eb + 1) * 128]
            wht_blocks[eb * KD + k] = whT[k * 128:(k + 1) * 128,
                                          eb * 128:(eb + 1) * 128]
    wa_c = np.ascontiguousarray(
        w_a.reshape(KD, 128).T).astype(ml_dtypes.bfloat16)
    ident = np.eye(128, dtype=np.float32)
    ones = np.ones((128, 2), np.float32)

    in_maps = []
    for i in range(N_CORES):
        sl = slice(i * B_LOC, (i + 1) * B_LOC)
        x_i = np.ascontiguousarray(
            full_input[sl].reshape(NTOK, D).astype(np.float32))
        enc_i = np.ascontiguousarray(
            encoding[sl].T.reshape(KD, 128, B_LOC).transpose(1, 0, 2)
            .reshape(128, KD * B_LOC)).astype(ml_dtypes.bfloat16)
        mask_i = np.ascontiguousarray(
            mask[sl].reshape(NTOK // 128, 128).T.astype(np.float32))
        in_maps.append({
            "x": x_i, "wyt": wyt_blocks, "wht": wht_blocks,
            "enc_cols": enc_i, "wa_cols": wa_c, "mask_cols": mask_i,
            "ident": ident, "ones": ones,
        })
    return in_maps


def run(inputs, trace=False):
    if "nc" not in _CACHE:
        _CACHE["nc"] = build()
    nc = _CACHE["nc"]
    in_maps = _host_pack(**inputs)
    res = run_bass_kernel_spmd(nc, in_maps, core_ids=list(range(N_CORES)),
                               trace=trace)
    out = np.concatenate([res.results[i]["out"] for i in range(N_CORES)],
                         axis=0)
    return out, res


def kernel(**inputs):
    out, _ = run(inputs, trace=False)
    return out
